# revision 1
# baseline (speedup 1.0000x reference)
"""MoE routing kernel (nn_MoE_52037823758984) for 8x Trainium2 NeuronCores.

Computes out[i] = expert_{route[i]}(x[i]) where each expert is a Linear(10,10):
    y0 = x @ W1.T + b1 ; y1 = x @ W2.T + b2 ; out = where(route==0, y0, y1)

Sharding: data-parallel over the token dim; each of the 8 cores processes
N/8 = 262144 tokens. The program is built at kernel() call time, when the
weights are known.

Shipped algorithm (build_moe_v3, r_tile=512, gp_tiles=0; token-major,
f32-exact):
    out = (delta masked by route) + expert-1,  via linearity:
    out_j = r * (sum_k x_k*Wd[j,k] + bd[j]) + sum_k x_k*W1[j,k] + b1[j]
  with Wd = W2-W1, bd = b2-b1, r = float(route).
  - x tile [128, R, 10] (partition = contiguous token block) is re-laid out
    feature-planar [128, 10, R] on the Scalar engine so every Vector-engine
    op streams contiguous [128, R] slices (2x fp32 perf mode);
  - weights enter as [128,1] SBUF access-pattern scalars (wt side input),
    not instruction immediates (immediates measured ~5x slower);
  - per output feature j on DVE: tensor_scalar init (Wd[j,0], bd[j]), 9
    scalar_tensor_tensor accumulates, 1 tensor_mul mask by r, 10 more
    accumulates for W1;
  - un-planarize is fused with the b1[j] bias add on the Scalar engine
    (ACTIVATE Identity with per-partition bias), staged in a tile that
    reuses the dead x-tile ring to fit R=512 at bufs=3.
Earlier variants (build_moe, build_moe_planar, gp_tiles>0) are kept for
reference; GPSIMD tensor ops measured ~18x slower per op than DVE here.
"""

import numpy as np

import concourse.bacc as bacc
import concourse.mybir as mybir
from concourse.tile import TileContext
from concourse.bass_utils import run_bass_kernel_spmd

F32 = mybir.dt.float32
I32 = mybir.dt.int32
ALU = mybir.AluOpType

N_CORES = 8
P = 128


def build_moe(tc_tokens, W1, b1, W2, b2, r_tile=256, reps=1):
    """Build + compile the per-core program for a shard of `tc_tokens` tokens."""
    D = 10
    Wd = (W2.astype(np.float64) - W1.astype(np.float64))
    bd = (b2.astype(np.float64) - b1.astype(np.float64))
    W1 = W1.astype(np.float64)
    b1 = b1.astype(np.float64)

    R = r_tile
    assert tc_tokens % (P * R) == 0
    nt = tc_tokens // (P * R)

    nc = bacc.Bacc("TRN2", target_bir_lowering=False, debug=False,
                   num_devices=N_CORES)
    x_ext = nc.dram_tensor("x", [tc_tokens, D], F32, kind="ExternalInput")
    r_ext = nc.dram_tensor("route", [tc_tokens], I32, kind="ExternalInput")
    o_ext = nc.dram_tensor("out", [tc_tokens, D], F32, kind="ExternalOutput")

    # partition p holds a contiguous run of R tokens
    xv = x_ext.rearrange("(n p r) d -> n p r d", p=P, r=R)
    rv = r_ext.rearrange("(n p r) -> n p r", p=P, r=R)
    ov = o_ext.rearrange("(n p r) d -> n p r d", p=P, r=R)

    with TileContext(nc) as tc:
        with tc.tile_pool(name="sbuf", bufs=2) as pool:
            for _ in range(reps):
                for i in range(nt):
                    xt = pool.tile([P, R, D], F32, tag="xt")
                    rt = pool.tile([P, R], I32, tag="rt")
                    nc.sync.dma_start(out=xt[:], in_=xv[i])
                    nc.sync.dma_start(out=rt[:], in_=rv[i])

                    rf = pool.tile([P, R], F32, tag="rf")
                    nc.vector.tensor_copy(out=rf[:], in_=rt[:])  # int->float

                    xm = pool.tile([P, R, D], F32, tag="xm")  # x * r
                    for k in range(D):
                        nc.vector.tensor_mul(out=xm[:, :, k], in0=xt[:, :, k],
                                             in1=rf[:])

                    acc = pool.tile([P, R, D], F32, tag="acc")
                    for j in range(D):
                        nc.vector.tensor_scalar(
                            out=acc[:, :, j], in0=xm[:, :, 0],
                            scalar1=float(Wd[j, 0]), scalar2=float(b1[j]),
                            op0=ALU.mult, op1=ALU.add)
                        for k in range(1, D):
                            nc.vector.scalar_tensor_tensor(
                                out=acc[:, :, j], in0=xm[:, :, k],
                                scalar=float(Wd[j, k]), in1=acc[:, :, j],
                                op0=ALU.mult, op1=ALU.add)
                        nc.vector.scalar_tensor_tensor(
                            out=acc[:, :, j], in0=rf[:],
                            scalar=float(bd[j]), in1=acc[:, :, j],
                            op0=ALU.mult, op1=ALU.add)
                        for k in range(D):
                            nc.vector.scalar_tensor_tensor(
                                out=acc[:, :, j], in0=xt[:, :, k],
                                scalar=float(W1[j, k]), in1=acc[:, :, j],
                                op0=ALU.mult, op1=ALU.add)
                    nc.sync.dma_start(out=ov[i], in_=acc[:])
    nc.compile()
    return nc


def build_moe_planar(tc_tokens, W1, b1, W2, b2, r_tile=256, reps=1):
    """Planar variant: all DVE ops on contiguous [128, R] slices; weights as
    [128,1] SBUF scalars (replicated via a small extra input) instead of
    per-instruction immediates.

    wt layout (cols): 0-99 Wd[j,k] at j*10+k; 100-199 W1[j,k]; 200-209 bd;
    210-219 b1.
    """
    D = 10
    R = r_tile
    assert tc_tokens % (P * R) == 0
    nt = tc_tokens // (P * R)

    nc = bacc.Bacc("TRN2", target_bir_lowering=False, debug=False,
                   num_devices=N_CORES)
    x_ext = nc.dram_tensor("x", [tc_tokens, D], F32, kind="ExternalInput")
    r_ext = nc.dram_tensor("route", [tc_tokens], I32, kind="ExternalInput")
    w_ext = nc.dram_tensor("wt", [P, 220], F32, kind="ExternalInput")
    o_ext = nc.dram_tensor("out", [tc_tokens, D], F32, kind="ExternalOutput")

    xv = x_ext.rearrange("(n p r) d -> n p r d", p=P, r=R)
    rv = r_ext.rearrange("(n p r) -> n p r", p=P, r=R)
    ov = o_ext.rearrange("(n p r) d -> n p r d", p=P, r=R)

    with TileContext(nc) as tc:
        with tc.tile_pool(name="const", bufs=1) as cpool, \
             tc.tile_pool(name="sbuf", bufs=2) as pool:
            wt = cpool.tile([P, 220], F32)
            nc.sync.dma_start(out=wt[:], in_=w_ext[:])

            def wd(j, k):
                return wt[:, j * 10 + k:j * 10 + k + 1]

            def w1(j, k):
                return wt[:, 100 + j * 10 + k:100 + j * 10 + k + 1]

            def bd(j):
                return wt[:, 200 + j:200 + j + 1]

            def b1(j):
                return wt[:, 210 + j:210 + j + 1]

            for _ in range(reps):
                for i in range(nt):
                    xt = pool.tile([P, R, D], F32, tag="xt")
                    rt = pool.tile([P, R], I32, tag="rt")
                    nc.sync.dma_start(out=xt[:], in_=xv[i])
                    nc.sync.dma_start(out=rt[:], in_=rv[i])

                    rf = pool.tile([P, R], F32, tag="rf")
                    nc.vector.tensor_copy(out=rf[:], in_=rt[:])

                    xp = pool.tile([P, D, R], F32, tag="xp")  # planar x
                    for k in range(D):
                        nc.vector.tensor_copy(out=xp[:, k, :], in_=xt[:, :, k])

                    accp = pool.tile([P, D, R], F32, tag="accp")
                    for j in range(D):
                        aj = accp[:, j, :]
                        nc.vector.tensor_scalar(
                            out=aj, in0=xp[:, 0, :], scalar1=wd(j, 0),
                            scalar2=bd(j), op0=ALU.mult, op1=ALU.add)
                        for k in range(1, D):
                            nc.vector.scalar_tensor_tensor(
                                out=aj, in0=xp[:, k, :], scalar=wd(j, k),
                                in1=aj, op0=ALU.mult, op1=ALU.add)
                        # mask the delta expert, then add expert-1 terms
                        nc.vector.tensor_mul(out=aj, in0=aj, in1=rf[:])
                        for k in range(D):
                            nc.vector.scalar_tensor_tensor(
                                out=aj, in0=xp[:, k, :], scalar=w1(j, k),
                                in1=aj, op0=ALU.mult, op1=ALU.add)
                        nc.vector.tensor_scalar_add(out=aj, in0=aj,
                                                    scalar1=b1(j))
                    # un-planarize and store
                    acc = pool.tile([P, R, D], F32, tag="acc")
                    for d in range(D):
                        nc.vector.tensor_copy(out=acc[:, :, d], in_=accp[:, d, :])
                    nc.sync.dma_start(out=ov[i], in_=acc[:])
    nc.compile()
    return nc


def build_moe_v3(tc_tokens, W1, b1, W2, b2, r_tile=256, reps=1, gp_tiles=2,
                 layout="new"):
    """v3: engine-split variant.

    - chain (the 210 multiply-accumulate ops/tile) runs on DVE for most tiles
      and on GPSIMD for `gp_tiles` of every 8, so the two engines work in
      parallel;
    - glue ops move to the Scalar engine (ACT): feature-planarize copies and
      the un-planarize which is fused with the per-feature bias add
      (ACTIVATE Copy with per-partition bias AP).
    """
    D = 10
    R = r_tile
    assert tc_tokens % (P * R) == 0
    nt = tc_tokens // (P * R)
    AF = mybir.ActivationFunctionType

    nc = bacc.Bacc("TRN2", target_bir_lowering=False, debug=False,
                   num_devices=N_CORES)
    x_ext = nc.dram_tensor("x", [tc_tokens, D], F32, kind="ExternalInput")
    r_ext = nc.dram_tensor("route", [tc_tokens], I32, kind="ExternalInput")
    w_ext = nc.dram_tensor("wt", [P, 220], F32, kind="ExternalInput")
    o_ext = nc.dram_tensor("out", [tc_tokens, D], F32, kind="ExternalOutput")

    xv = x_ext.rearrange("(n p r) d -> n p r d", p=P, r=R)
    rv = r_ext.rearrange("(n p r) -> n p r", p=P, r=R)
    ov = o_ext.rearrange("(n p r) d -> n p r d", p=P, r=R)

    # spread the gpsimd-chain tiles evenly through the loop
    gp_set = set()
    if gp_tiles > 0:
        stride = max(1, nt // gp_tiles)
        gp_set = {i for i in range(nt) if i % stride == stride - 1}
        while len(gp_set) > gp_tiles:
            gp_set.pop()

    with TileContext(nc) as tc:
        n_bufs = (4 if R <= 256 else 3) if layout == 'new' else 3
        with tc.tile_pool(name="const", bufs=1) as cpool, \
             tc.tile_pool(name="sbuf", bufs=n_bufs) as pool:
            wt = cpool.tile([P, 220], F32)
            nc.sync.dma_start(out=wt[:], in_=w_ext[:])

            def ap_wd(j, k):
                return wt[:, j * 10 + k:j * 10 + k + 1]

            def ap_w1(j, k):
                return wt[:, 100 + j * 10 + k:100 + j * 10 + k + 1]

            def ap_bd(j):
                return wt[:, 200 + j:200 + j + 1]

            def ap_b1(j):
                return wt[:, 210 + j:210 + j + 1]

            for _ in range(reps):
                for i in range(nt):
                    eng = nc.gpsimd if i in gp_set else nc.vector
                    xt = pool.tile([P, R, D], F32, tag="xt")
                    rt = pool.tile([P, R], I32,
                                   tag="rtf" if layout == "new" else "rt")
                    nc.sync.dma_start(out=xt[:], in_=xv[i])
                    nc.sync.dma_start(out=rt[:], in_=rv[i])

                    rf = pool.tile([P, R], F32,
                                   tag="rtf" if layout == "new" else "rf")
                    eng.tensor_copy(out=rf[:], in_=rt[:])

                    xp = pool.tile([P, D, R], F32, tag="xp")
                    for k in range(D):
                        nc.scalar.copy(out=xp[:, k, :], in_=xt[:, :, k])

                    is_gp = i in gp_set
                    Wdv = W2.astype(np.float64) - W1.astype(np.float64)
                    bdv = b2.astype(np.float64) - b1.astype(np.float64)

                    def s_wd(j, k):
                        return float(Wdv[j, k]) if is_gp else ap_wd(j, k)

                    def s_w1(j, k):
                        return float(W1[j, k]) if is_gp else ap_w1(j, k)

                    def s_bd(j):
                        return float(bdv[j]) if is_gp else ap_bd(j)

                    accp = pool.tile([P, D, R], F32, tag="accp")
                    if is_gp:
                        tmp = pool.tile([P, R], F32, tag="gptmp")
                    for j in range(D):
                        aj = accp[:, j, :]
                        if is_gp:
                            # Pool engine has no fused scalar_tensor_tensor;
                            # use mul + add pairs with float immediates.
                            eng.tensor_scalar_mul(out=aj, in0=xp[:, 0, :],
                                                  scalar1=s_wd(j, 0))
                            eng.tensor_scalar_add(out=aj, in0=aj,
                                                  scalar1=s_bd(j))
                            for k in range(1, D):
                                eng.tensor_scalar_mul(out=tmp[:], in0=xp[:, k, :],
                                                      scalar1=s_wd(j, k))
                                eng.tensor_add(out=aj, in0=aj, in1=tmp[:])
                            eng.tensor_mul(out=aj, in0=aj, in1=rf[:])
                            for k in range(D):
                                eng.tensor_scalar_mul(out=tmp[:], in0=xp[:, k, :],
                                                      scalar1=s_w1(j, k))
                                eng.tensor_add(out=aj, in0=aj, in1=tmp[:])
                        else:
                            eng.tensor_scalar(
                                out=aj, in0=xp[:, 0, :], scalar1=s_wd(j, 0),
                                scalar2=s_bd(j), op0=ALU.mult, op1=ALU.add)
                            for k in range(1, D):
                                eng.scalar_tensor_tensor(
                                    out=aj, in0=xp[:, k, :], scalar=s_wd(j, k),
                                    in1=aj, op0=ALU.mult, op1=ALU.add)
                            eng.tensor_mul(out=aj, in0=aj, in1=rf[:])
                            for k in range(D):
                                eng.scalar_tensor_tensor(
                                    out=aj, in0=xp[:, k, :], scalar=s_w1(j, k),
                                    in1=aj, op0=ALU.mult, op1=ALU.add)
                    # un-planarize fused with bias add on ACT; reuse the
                    # xt ring (xt is dead once planarized)
                    acc = pool.tile([P, R, D], F32,
                                    tag="xt" if layout == "new" else "acc")
                    for j in range(D):
                        nc.scalar.activation(out=acc[:, :, j], in_=accp[:, j, :],
                                             func=AF.Identity, bias=ap_b1(j),
                                             scale=1.0)
                    nc.sync.dma_start(out=ov[i], in_=acc[:])
    nc.compile()
    return nc


def make_wt(W1, b1, W2, b2):
    Wd = (W2 - W1)
    bdv = (b2 - b1)
    cols = np.concatenate([Wd.reshape(-1), W1.reshape(-1), bdv, b1]).astype(np.float32)
    return np.tile(cols[None, :], (P, 1))


def run_sharded(nc, x, route, tc_tokens, wt=None, out_name="out"):
    in_maps = []
    for c in range(N_CORES):
        sl = slice(c * tc_tokens, (c + 1) * tc_tokens)
        m = {"x": np.ascontiguousarray(x[sl]),
             "route": np.ascontiguousarray(route[sl])}
        if wt is not None:
            m["wt"] = wt
        in_maps.append(m)
    res = run_bass_kernel_spmd(nc, in_maps, core_ids=list(range(N_CORES)))
    return np.concatenate([res.results[c][out_name] for c in range(N_CORES)],
                          axis=0)


def kernel(x, W1, b1, W2, b2, route):
    x = np.asarray(x)
    route = np.asarray(route)
    W1, b1 = np.asarray(W1), np.asarray(b1)
    W2, b2 = np.asarray(W2), np.asarray(b2)
    tc_tokens = x.shape[0] // N_CORES
    # v3 with gp_tiles=0 == DVE chain + Scalar-engine glue; fastest measured
    # config in same-process A/B: R=512 tiles, output staging reusing the xt
    # ring, bufs=3
    nc = build_moe_v3(tc_tokens, W1, b1, W2, b2, r_tile=512, gp_tiles=0)
    return run_sharded(nc, x, route, tc_tokens, wt=make_wt(W1, b1, W2, b2))



# revision 2
# speedup vs baseline: 1.7706x; 1.7706x over previous
"""MoE routing kernel v2 (Plan Z): TensorEngine planar pipeline.

out = y1 + r*yd,  y1 = x@W1.T+b1, yd = x@Wd.T+bd, Wd=W2-W1, r=route.

Per-core layout: tokens t = p*2048 + i*R + l  (p partition, i tile, l low).
 xr [128, R, 11] f32: cols 0-9 = x features, col 10 = r  (built on-chip).
 Chunks of 11 lows (121 free cols); PE-transpose (fp32 fast mode) to planar
 psum rows 11g+f, plus a ones-row at 121.  Three bf16 matmuls vs the same
 moving x_pl: S_A -> y1 (+b1 via ones-row), S_B -> yd (+bd), S_R -> r_pl
 (replicates each group's r-row across its 10 output rows).  DVE 2-op
 select in planar, fp32 PE back-transpose, DVE drains token-major psum
 into the f32 out tile.
"""

import numpy as np

import concourse.bacc as bacc
import concourse.mybir as mybir
from concourse.tile import TileContext
from concourse.masks import make_identity
from concourse.bass_utils import run_bass_kernel_spmd

F32 = mybir.dt.float32
BF16 = mybir.dt.bfloat16
I32 = mybir.dt.int32

N_CORES = 8
P = 128
D = 10
DF = 11            # features incl. r column
KK = 121           # planar rows: 11 groups * 11
MM = 110           # out rows: 11 groups * 10


def pack_wt(W1, b1, W2, b2):
    """[128, 662] f32: cols 0:110 S_A, 110:220 S_B, 220:330 S_R,
    330 = bd col, 331 = b1 col, 332:442 S_A ragged(6 groups),
    442:552 S_B ragged, 552:662 S_R ragged."""
    Wd = W2.astype(np.float64) - W1.astype(np.float64)
    bd = b2.astype(np.float64) - b1.astype(np.float64)
    out = np.zeros((P, 662), np.float32)

    def fill(dst, Wm, rrow, groups=11):
        for g in range(groups):
            for u in range(D):
                if rrow:
                    dst[DF * g + D, D * g + u] = 1.0
                else:
                    for k in range(D):
                        dst[DF * g + k, D * g + u] = Wm[u, k]

    for base, gn in ((0, 11), (332, 6)):
        SA = np.zeros((P, MM), np.float64); fill(SA, W1, False, gn)
        SB = np.zeros((P, MM), np.float64); fill(SB, Wd, False, gn)
        SR = np.zeros((P, MM), np.float64); fill(SR, None, True, gn)
        out[:, base:base + 110] = SA
        out[:, base + 110:base + 220] = SB
        out[:, base + 220:base + 330] = SR
    out[:MM, 330] = np.tile(bd, 11)
    out[:MM, 331] = np.tile(b1, 11)
    return out


def build_moe_pe(tc_tokens, r_tile=512, reps=1, drains=("vector", "scalar",
                                                        "scalar"),
                 pbufs=(3, 2, 1), ileave="vector", stage=3, dma_ileave=False):
    """drains = engines for (x_pl copy, r_pl copy, final out copy)."""
    R = r_tile
    assert tc_tokens % (P * R) == 0
    nt = tc_tokens // (P * R)
    FR = R * D           # f32 free cols per partition of x / out tiles
    FRX = R * DF         # free cols of interleaved xr
    NCH = FRX // (11 * DF)          # full 11-low chunks per tile
    rag_lows = R - NCH * 11         # leftover lows
    
    nc = bacc.Bacc("TRN2", target_bir_lowering=False, debug=False,
                   num_devices=N_CORES)
    x_ext = nc.dram_tensor("x", [tc_tokens, D], F32, kind="ExternalInput")
    r_ext = nc.dram_tensor("route", [tc_tokens], I32, kind="ExternalInput")
    w_ext = nc.dram_tensor("wt", [P, 662], F32, kind="ExternalInput")
    o_ext = nc.dram_tensor("out", [tc_tokens, D], F32, kind="ExternalOutput")

    xv = x_ext.rearrange("(n p r) d -> n p (r d)", p=P, r=R)
    xvs = x_ext.rearrange("(n p r) d -> n p r d", p=P, r=R)
    rv = r_ext.rearrange("(n p r) -> n p r", p=P, r=R)
    ov = o_ext.rearrange("(n p r) d -> n p (r d)", p=P, r=R)

    # chunk groups of up to 4 full chunks, then one ragged group
    groups = []
    c = 0
    while c < NCH:
        ng = min(4, NCH - c)
        groups.append((c, ng))
        c += ng

    with TileContext(nc) as tc:
        with tc.tile_pool(name="const", bufs=1) as cpool, \
             tc.tile_pool(name="sbuf", bufs=3) as pool, \
             tc.tile_pool(name="sbo", bufs=2) as pool2, \
             tc.psum_pool(name="ppx", bufs=pbufs[0]) as ppx, \
             tc.psum_pool(name="ppm", bufs=1) as ppm:
            def drain_copy(which, out, in_):
                if which == "scalar":
                    nc.scalar.copy(out=out, in_=in_)
                else:
                    nc.vector.tensor_copy(out=out, in_=in_)
            idf = cpool.tile([P, P], F32)
            make_identity(nc, idf[:])

            wt = cpool.tile([P, 662], F32)
            nc.sync.dma_start(out=wt[:], in_=w_ext[:])
            SA = cpool.tile([KK, MM], BF16)
            nc.vector.tensor_copy(out=SA[:], in_=wt[:KK, 0:110])
            SB = cpool.tile([KK, MM], BF16)
            nc.vector.tensor_copy(out=SB[:], in_=wt[:KK, 110:220])
            SR = cpool.tile([KK, MM], BF16)
            nc.vector.tensor_copy(out=SR[:], in_=wt[:KK, 220:330])
            SAr = cpool.tile([KK, MM], BF16)
            nc.vector.tensor_copy(out=SAr[:], in_=wt[:KK, 332:442])
            SBr = cpool.tile([KK, MM], BF16)
            nc.vector.tensor_copy(out=SBr[:], in_=wt[:KK, 442:552])
            SRr = cpool.tile([KK, MM], BF16)
            nc.vector.tensor_copy(out=SRr[:], in_=wt[:KK, 552:662])

            for _ in range(reps):
                stageB = []   # deferred per-group closures (one iter later)
                stageC = []   # two iters later

                def run_deferred(lists):
                    for q in lists:
                        while q:
                            q.pop(0)()

                for i in range(nt):
                    xt = pool.tile([P, FR], F32, tag="xt", bufs=2)
                    nc.sync.dma_start(out=xt[:], in_=xv[i])
                    rt = pool.tile([P, R], I32, tag="rt")
                    nc.sync.dma_start(out=rt[:], in_=rv[i])
                    xr = pool.tile([P, R, DF], F32, tag="xr", bufs=2)
                    xtv = xt[:].rearrange("p (r d) -> p r d", d=D)
                    if ileave == "scalar":
                        nc.scalar.copy(out=xr[:, :, 0:D], in_=xtv[:])
                    elif ileave == "split":
                        h = R // 2
                        nc.scalar.copy(out=xr[:, :h, 0:D], in_=xtv[:, :h])
                        nc.vector.tensor_copy(out=xr[:, h:, 0:D],
                                              in_=xtv[:, h:])
                    else:
                        nc.vector.tensor_copy(out=xr[:, :, 0:D], in_=xtv[:])
                    nc.vector.tensor_copy(out=xr[:, :, D], in_=rt[:])
                    xrf = xr[:].rearrange("p r d -> p (r d)")

                    out_tile = pool2.tile([P, FR], F32, tag="ot")
                    glist = groups + ([(NCH, 0)] if rag_lows else [])
                    last_gi = len(glist) - 1

                    for gi, (c0, ng) in enumerate(glist):
                        rag = ng == 0
                        N = 128 * (ng if not rag else 1)
                        nc4 = ng if not rag else 1
                        # ---- stage A (group g): transposes, drain0, mms
                        ps_x = ppx.tile([P, 512], F32, tag="pst")
                        for k in range(nc4):
                            ch = c0 + k
                            fl = 11 * DF if not rag else rag_lows * DF
                            nc.tensor.transpose(
                                ps_x[0:fl, 128 * k:128 * k + 128],
                                xrf[:, 121 * ch:121 * ch + fl], idf[:])
                        x_pl = pool.tile([KK, 512], BF16, tag="xpl")
                        d0 = drains[0] if drains[0] != "mix" else \
                            ("scalar" if gi % 2 == 0 else "vector")
                        drain_copy(d0, x_pl[:, :N], ps_x[:KK, :N])
                        ps_ab = ppm.tile([MM, 1024], F32, tag="psab",
                                         bufs=pbufs[1])
                        ps_r = ppm.tile([MM, 512], F32, tag="psr",
                                        bufs=pbufs[2])
                        Sa, Sb, Sr = (SA, SB, SR) if not rag else \
                            (SAr, SBr, SRr)
                        nc.tensor.matmul(ps_r[:, 0:N], Sr[:], x_pl[:, :N])
                        nc.tensor.matmul(ps_ab[:, 0:N], Sa[:], x_pl[:, :N])
                        nc.tensor.matmul(ps_ab[:, 512:512 + N], Sb[:],
                                         x_pl[:, :N])

                        # ---- deferred stage B for this group
                        def mk_stageB(ps_ab=ps_ab, ps_r=ps_r, N=N, nc4=nc4,
                                      c0=c0, rag=rag, out_tile=out_tile, gi=gi):
                            def f():
                                ALU = mybir.AluOpType
                                r_pl = pool.tile([MM, 512], F32, tag="rpl")
                                d1 = drains[1] if drains[1] != "mix" else \
                                    ("vector" if gi % 2 == 0 else "scalar")
                                drain_copy(d1, r_pl[:, :N], ps_r[:, :N])
                                m_sb = pool.tile([MM, 512], F32, tag="msb")
                                nc.vector.scalar_tensor_tensor(
                                    out=m_sb[:, :N],
                                    in0=ps_ab[:, 512:512 + N],
                                    scalar=wt[0:MM, 330:331],
                                    in1=r_pl[:, :N],
                                    op0=ALU.add, op1=ALU.mult)
                                out_pl = pool.tile([P, 512], F32, tag="opl")
                                nc.vector.scalar_tensor_tensor(
                                    out=out_pl[:MM, :N], in0=ps_ab[:, 0:N],
                                    scalar=wt[0:MM, 331:332],
                                    in1=m_sb[:, :N],
                                    op0=ALU.add, op1=ALU.add)
                                ps_o = ppx.tile([P, 512], F32, tag="pst")
                                for k in range(nc4):
                                    nc.tensor.transpose(
                                        ps_o[:, 128 * k:128 * k + 128],
                                        out_pl[:, 128 * k:128 * k + 128],
                                        idf[:])
                                # deferred stage C: final drain
                                def g():
                                    nlow = 11 if not rag else rag_lows
                                    KB = nlow * D
                                    ob = 110 * c0
                                    wid = 110 * nc4 if not rag else KB
                                    pov = ps_o[:].rearrange(
                                        "p (k c) -> p k c", c=128)
                                    d2 = drains[2] if drains[2] != "mix" \
                                        else ("scalar" if gi % 2 == 0
                                              else "vector")
                                    drain_copy(d2,
                                               out_tile[:, ob:ob + wid],
                                               pov[:, :nc4, :KB])
                                stageC.append(g)
                            return f
                        stageB.append(mk_stageB())

                        # run one deferred closure from each older stage
                        if stageC:
                            stageC.pop(0)()
                        if len(stageB) > 1:
                            stageB.pop(0)()

                    def mk_dma(i=i, out_tile=out_tile):
                        def f():
                            nc.sync.dma_start(out=ov[i], in_=out_tile[:])
                        return f
                    if i == nt - 1:
                        run_deferred([stageB, stageC])
                        mk_dma()()
                    else:
                        # emitted via B-queue so it lands after this tile's
                        # last stage-C drain in the C-queue
                        stageB.append(
                            lambda f=mk_dma(): stageC.append(f))
    nc.compile()
    return nc


def run_sharded(nc, x, route, tc_tokens, wt):
    in_maps = []
    for c in range(N_CORES):
        sl = slice(c * tc_tokens, (c + 1) * tc_tokens)
        in_maps.append({"x": np.ascontiguousarray(x[sl]),
                        "route": np.ascontiguousarray(route[sl]),
                        "wt": wt})
    res = run_bass_kernel_spmd(nc, in_maps, core_ids=list(range(N_CORES)))
    return np.concatenate([res.results[c]["out"] for c in range(N_CORES)],
                          axis=0)


def kernel(x, W1, b1, W2, b2, route):
    x = np.asarray(x)
    route = np.asarray(route)
    tc_tokens = x.shape[0] // N_CORES
    nc = build_moe_pe(tc_tokens, r_tile=512)
    return run_sharded(nc, x, route, tc_tokens,
                       wt=pack_wt(np.asarray(W1), np.asarray(b1),
                                  np.asarray(W2), np.asarray(b2)))
